# revision 18
# baseline (speedup 1.0000x reference)
# Trainium2 Bass kernel for nn_Decoder: embedding -> 2 residual LSTM layers ->
# 2x additive attention -> LSTM -> 32000-vocab projection.
#
# Strategy (8 NeuronCores, wavefront pipeline over 8 sequence chunks of 16 steps):
#   core 0: LSTM layer 1   core 1: LSTM layer 2   core 2: LSTM layer 3
#   cores 3-7: additive attention (sharded by (attn, batch) "pab" units)
#   fc (vocab projection): all 8 cores, vocab-sharded, after the pipeline.
# Chunk handoffs via per-tick AllGather through internal DRAM.
import numpy as np
import ml_dtypes

Tq, Tk, B, D, V = 128, 256, 8, 512, 32000
H4 = 2048
NCH, CS = 8, 16          # chunks x steps-per-chunk
SB = CS * B              # 128 columns per chunk
NT = NCH + 3             # 11 ticks
SLOTF = 128 * 256        # floats per rank slot
VS = 4096                # per-core vocab shard (8 x 512-wide blocks)
NVB = [8] * 8
OFFS = [c * VS for c in range(8)]
BF = ml_dtypes.bfloat16

# gate tile order: g, i, f, o  (PyTorch row order is i, f, g, o)
GPERM = np.concatenate([np.arange(1024, 1536), np.arange(0, 512),
                        np.arange(512, 1024), np.arange(1536, 2048)])
# slices in tile units (16 tiles of 128 gates)
GT_G, GT_I, GT_F, GT_O = slice(0, 4), slice(4, 8), slice(8, 12), slice(12, 16)

def _pab_table():
    # per attn core (idx 0..4): 4 slots of (attn, b) or None (dummy)
    t = []
    for k in range(4):
        ids = [3 * k, 3 * k + 1, 3 * k + 2]
        t.append([(i // 8, i % 8) for i in ids] + [None])
    t.append([(i // 8, i % 8) for i in (12, 13, 14, 15)])
    return t
PABS = _pab_table()

def _wt_tiles(WT, nct):
    # WT: (in_dim, H4) -> (nct, 128, 16, 128) [ct, p, gt, gp]
    ind = WT.shape[0]
    out = np.zeros((nct, 128, 16, 128), np.float32)
    w = WT.reshape(ind // 128, 128, 16, 128)
    out[: ind // 128] = w
    return out.astype(BF)

def host_prep(inp):
    """Build the 8 per-core input dicts."""
    f32 = np.float32
    tok = np.asarray(inp["inputs"]).astype(np.int64)          # (Tq, B)
    emb = np.asarray(inp["emb"], f32)
    x1 = emb[tok.reshape(-1)]                                  # (Tq*B, D) col = s*B+b
    x_src = np.ascontiguousarray(x1.T.reshape(4, 128, Tq * B)).astype(f32)

    def lstm_pack(Wih, Whh, bih, bhh, h0, c0):
        ind = Wih.shape[1]
        WihP = Wih[GPERM]                                      # (H4, ind)
        WhhP = Whh[GPERM]
        d = {}
        d["wih_t"] = np.zeros((8, 128, 16, 128), BF)
        d["wih_t"][: ind // 128] = _wt_tiles(np.ascontiguousarray(WihP.T), ind // 128)
        d["whh_t"] = _wt_tiles(np.ascontiguousarray(WhhP.T), 4)
        d["gbias_r"] = ((bih + bhh)[GPERM]).reshape(1, 16, 128).astype(BF)
        hc = np.zeros((2, 128, 4, 8), f32)
        hc[0] = h0.T.reshape(4, 128, 8).transpose(1, 0, 2)     # (p, ct, b)
        hc[1] = c0.T.reshape(4, 128, 8).transpose(1, 0, 2)
        d["hc0"] = hc
        return d

    lay = []
    for l in range(2):
        lay.append(lstm_pack(np.asarray(inp["Wih_res"], f32)[l], np.asarray(inp["Whh_res"], f32)[l],
                             np.asarray(inp["bih_res"], f32)[l], np.asarray(inp["bhh_res"], f32)[l],
                             np.asarray(inp["h0"], f32)[l], np.asarray(inp["c0"], f32)[l]))
    lay.append(lstm_pack(np.asarray(inp["WihF"], f32), np.asarray(inp["WhhF"], f32),
                         np.asarray(inp["bihF"], f32), np.asarray(inp["bhhF"], f32),
                         np.asarray(inp["h0"], f32)[2], np.asarray(inp["c0"], f32)[2]))
    zlay = {k: np.zeros_like(v) for k, v in lay[0].items()}

    # shared attention tensors (Qw both attns)
    qw2 = np.zeros((2, 4, 128, 4, 128), BF)
    qb2 = np.zeros((128, 2, 4), f32)
    for a in range(2):
        Qw = np.asarray(inp["Qw%d" % (a + 1)], f32)            # (D, D) a_out x d
        qw2[a] = np.ascontiguousarray(Qw.T).reshape(4, 128, 4, 128).astype(BF)
        qb2[:, a, :] = np.asarray(inp["Qb%d" % (a + 1)], f32).reshape(4, 128).T
    ench = [np.asarray(inp["enc1"], f32), np.asarray(inp["enc2"], f32)]   # (Tk,B,D)
    maskh = [np.asarray(inp["mask1"]), np.asarray(inp["mask2"])]          # (Tq,Tk,B)

    def attn_pack(pabs):
        d = {}
        d["akw_t"] = np.zeros((4, 4, 128, 4, 128), BF)
        d["avw_t"] = np.zeros((4, 4, 128, 4, 128), BF)
        d["akb"] = np.zeros((4, 128, 4), f32)
        d["avb"] = np.zeros((4, 1, 512), BF)
        d["aenc_t"] = np.zeros((4, 4, 128, 256), BF)
        d["amask"] = np.ones((4, 128, 2, 128), BF)
        d["aww"] = np.zeros((4, 128, 4), BF)
        for i, pab in enumerate(pabs):
            if pab is None:
                continue
            a, b = pab
            Kw = np.asarray(inp["Kw%d" % (a + 1)], f32)
            Vw = np.asarray(inp["Vw%d" % (a + 1)], f32)
            d["akw_t"][i] = np.ascontiguousarray(Kw.T).reshape(4, 128, 4, 128).astype(BF)
            d["avw_t"][i] = np.ascontiguousarray(Vw.T).reshape(4, 128, 4, 128).astype(BF)
            d["akb"][i] = np.asarray(inp["Kb%d" % (a + 1)], f32).reshape(4, 128).T
            d["avb"][i, 0] = np.asarray(inp["Vb%d" % (a + 1)], f32)
            d["aenc_t"][i] = np.ascontiguousarray(ench[a][:, b, :].T).reshape(4, 128, 256).astype(BF)
            d["amask"][i] = np.ascontiguousarray(
                maskh[a][:, :, b].T.reshape(2, 128, 128).transpose(1, 0, 2)).astype(BF)
            d["aww"][i] = np.asarray(inp["Ww%d" % (a + 1)], f32)[0].reshape(4, 128).T.astype(BF)
        return d
    zattn = attn_pack([None] * 4)

    fcw = np.asarray(inp["fcw"], f32)
    fcwp = np.zeros((32768, D), f32)
    fcwp[:V] = fcw
    zx = np.zeros_like(x_src)

    cores = []
    for c in range(8):
        d = {}
        d["x_src"] = x_src if c == 0 else zx
        d.update(lay[c] if c < 3 else zlay)
        d.update(attn_pack(PABS[c - 3]) if c >= 3 else zattn)
        d["qw2_t"] = qw2 if c >= 3 else np.zeros_like(qw2)
        d["qb2"] = qb2 if c >= 3 else np.zeros_like(qb2)
        d["fcw_t"] = np.ascontiguousarray(
            fcwp[OFFS[c]:OFFS[c] + VS].T).reshape(4, 128, VS).astype(BF)
        cores.append(d)
    return cores

def host_post(results, inp):
    fcb = np.asarray(inp["fcb"], np.float32)
    out = np.zeros((Tq * B, 32768), np.float32)
    for c in range(8):
        w = NVB[c] * 512
        out[:, OFFS[c]:OFFS[c] + w] = np.asarray(
            results[c]["y"], np.float32).reshape(Tq * B, VS)[:, :w]
    return out[:, :V].reshape(Tq, B, V) + fcb[None, None, :]

# ----------------------------------------------------------------- device ---
_CACHE = {}

def build_kernel(stages=("l1", "l2", "l3", "attn", "fc"), dbg=False):
    key = (tuple(stages), dbg)
    if key in _CACHE:
        return _CACHE[key]
    import concourse.bacc as bacc
    import concourse.mybir as mybir
    from concourse.tile import TileContext
    from contextlib import ExitStack

    F32, BF16 = mybir.dt.float32, mybir.dt.bfloat16
    AF = mybir.ActivationFunctionType
    nc = bacc.Bacc("TRN2", target_bir_lowering=False, debug=False, num_devices=8)

    di = {}
    for name, shape, dt in [
        ("x_src", (4, 128, Tq * B), F32), ("wih_t", (8, 128, 16, 128), BF16),
        ("whh_t", (4, 128, 16, 128), BF16), ("gbias_r", (1, 16, 128), BF16),
        ("hc0", (2, 128, 4, 8), F32), ("akw_t", (4, 4, 128, 4, 128), BF16),
        ("avw_t", (4, 4, 128, 4, 128), BF16), ("akb", (4, 128, 4), F32),
        ("avb", (4, 1, 512), BF16), ("aenc_t", (4, 4, 128, 256), BF16),
        ("amask", (4, 128, 2, 128), BF16), ("aww", (4, 128, 4), BF16),
        ("qw2_t", (2, 4, 128, 4, 128), BF16), ("qb2", (128, 2, 4), F32),
        ("fcw_t", (4, 128, VS), BF16),
    ]:
        di[name] = nc.dram_tensor(name, list(shape), dt, kind="ExternalInput")
    y = nc.dram_tensor("y", [Tq * B, VS], BF16, kind="ExternalOutput")
    dbgout = nc.dram_tensor("dbgout", [NT, 8, SLOTF], F32, kind="ExternalOutput") if dbg else None
    cc_in = nc.dram_tensor("cc_in", [NT, SLOTF], F32)
    cc_out = nc.dram_tensor("cc_out", [NT, 8, SLOTF], F32)

    with TileContext(nc) as tc, ExitStack() as ctx:
        ET = mybir.EngineType
        pid = nc.partition_id(engines=(ET.PE, ET.Activation, ET.DVE, ET.SP, ET.Pool))
        pidc = nc.partition_id(engines=(ET.PE, ET.Activation, ET.DVE, ET.SP))
        pidv = nc.partition_id(engines=(ET.DVE,))
        P = lambda name, bufs, **kw: ctx.enter_context(tc.tile_pool(name=name, bufs=bufs, **kw))
        wp = P("wts", 1)
        # resident weights / constants
        wih = wp.tile([128, 8, 16, 128], BF16)
        nc.sync.dma_start(out=wih[:], in_=di["wih_t"].rearrange("c p g q -> p c g q"))
        whh = wp.tile([128, 4, 16, 128], BF16)
        nc.sync.dma_start(out=whh[:], in_=di["whh_t"].rearrange("c p g q -> p c g q"))
        gbias = wp.tile([1, 16, 128], BF16)
        nc.sync.dma_start(out=gbias[:], in_=di["gbias_r"][:])
        hc0 = wp.tile([128, 2, 4, 8], F32)
        nc.sync.dma_start(out=hc0[:], in_=di["hc0"].rearrange("h p c b -> p h c b"))
        qw2 = wp.tile([128, 2, 4, 4, 128], BF16)
        nc.sync.dma_start(out=qw2[:], in_=di["qw2_t"].rearrange("a c p t q -> p a c t q"))
        qb2 = wp.tile([128, 2, 4], F32)
        nc.sync.dma_start(out=qb2[:], in_=di["qb2"][:])
        amask = wp.tile([128, 4, 2, 128], BF16)
        nc.sync.dma_start(out=amask[:], in_=di["amask"].rearrange("i p k q -> p i k q"))
        aww = wp.tile([128, 4, 4], BF16)
        nc.sync.dma_start(out=aww[:], in_=di["aww"].rearrange("i p t -> p i t"))
        akb = wp.tile([128, 4, 4], F32)
        nc.sync.dma_start(out=akb[:], in_=di["akb"].rearrange("i p t -> p i t"))
        fcw = wp.tile([128, 4, VS], BF16)
        ones_r = wp.tile([1, 128], BF16)
        nc.vector.memset(ones_r[:], 1.0)
        ones_rf = wp.tile([1, 128], F32)
        nc.vector.memset(ones_rf[:], 1.0)
        ones_c = wp.tile([128, 1], BF16)
        nc.vector.memset(ones_c[:], 1.0)
        # persistent state
        h_f32 = wp.tile([128, 4, 8], F32)
        h_bf = wp.tile([128, 4, 8], BF16)
        c_st = wp.tile([128, 4, 8], F32)
        nc.vector.tensor_copy(h_f32[:], hc0[:, 0])
        nc.vector.tensor_copy(h_bf[:], hc0[:, 0])
        nc.vector.tensor_copy(c_st[:], hc0[:, 1])
        kp = wp.tile([128, 4, 4, 256], BF16)     # [p, slot, at, k]
        vp = wp.tile([128, 4, 2, 512], BF16)     # [p(k), slot, kb, at*128+ap]
        t3bf = wp.tile([128, 4, Tq * B], BF16)   # fc rhs accumulated

        # ---- warmup: kp/vp per pab slot (attention cores only) ----
        if "attn" in stages:
            with ExitStack() as wctx:
                wpool = wctx.enter_context(tc.tile_pool(name="warm", bufs=1))
                wps = wctx.enter_context(tc.tile_pool(name="warmps", bufs=1, space="PSUM"))
                akw = wpool.tile([128, 4, 4, 4, 128], BF16)
                avw = wpool.tile([128, 4, 4, 4, 128], BF16)
                aenc = wpool.tile([128, 4, 4, 256], BF16)
                avb = wpool.tile([1, 4, 512], BF16)
                with tc.If(pidc > 2):
                    nc.sync.dma_start(out=akw[:], in_=di["akw_t"].rearrange("i c p t q -> p i c t q"))
                    nc.sync.dma_start(out=avw[:], in_=di["avw_t"].rearrange("i c p t q -> p i c t q"))
                    nc.sync.dma_start(out=aenc[:], in_=di["aenc_t"].rearrange("i c p k -> p i c k"))
                    nc.sync.dma_start(out=avb[:], in_=di["avb"].rearrange("i o a -> o i a"))
                for i in range(4):
                    kpq = wps.tile([128, 4, 256], F32)
                    vpq = wps.tile([128, 512], F32)
                    with tc.If(pidc > 2):
                        for at in range(4):
                            for ct in range(4):
                                nc.tensor.matmul(kpq[:, at, :], akw[:, i, ct, at, :],
                                                 aenc[:, i, ct, :], start=(ct == 0), stop=(ct == 3))
                        for at in range(4):
                            nc.scalar.activation(kp[:, i, at, :], kpq[:, at, :],
                                                 AF.Identity, bias=akb[:, i, at:at + 1])
                        for kb in range(2):
                            nc.tensor.matmul(vpq[:], ones_r[:], avb[:, i, :], start=True, stop=False)
                            for ct in range(4):
                                nc.tensor.matmul(vpq[:], aenc[:, i, ct, kb * 128:(kb + 1) * 128],
                                                 avw[:, i, ct].rearrange("p t q -> p (t q)"),
                                                 start=False, stop=(ct == 3))
                            nc.vector.tensor_copy(vp[:, i, kb, :], vpq[:])

        nc.sync.dma_start(out=fcw[:], in_=di["fcw_t"].rearrange("c p v -> p c v"))
        # ------------------------------------------------ pipeline pools ----
        with ExitStack() as tctx:
            TP = lambda name, bufs, **kw: tctx.enter_context(tc.tile_pool(name=name, bufs=bufs, **kw))
            gps_pool = TP("gps", 2, space="PSUM")
            qpps_pool = TP("qpps", 1, space="PSUM")
            pwctx_pool = TP("pwctx", 1, space="PSUM")
            fcps_pool = TP("fcps", 2, space="PSUM")
            fcsb_pool = TP("fcsb", 3)
            lw_pool = TP("lwork", 2)
            st_pool = TP("stage", 2)
            gw_pool = TP("gwork", 3)
            aw_pool = TP("awork", 3)

            cco = cc_out.rearrange("t r (p x) -> t r p x", p=128)

            _fcn = [0]
            def emit_fc_vb(jf, vb, cpy):
                _fcn[0] += 1
                fp = fcps_pool.tile([128, 512], F32, tag="fp", name="fp%d" % _fcn[0])
                for ct in range(4):
                    nc.tensor.matmul(fp[:], t3bf[:, ct, jf * SB:jf * SB + 128],
                                     fcw[:, ct, vb * 512:(vb + 1) * 512],
                                     start=(ct == 0), stop=(ct == 3))
                ys = fcsb_pool.tile([128, 512], BF16, tag="ys", name="ys%d" % _fcn[0])
                cpy(ys[:], fp[:])
                nc.sync.dma_start(out=y[jf * 128:(jf + 1) * 128,
                                        vb * 512:(vb + 1) * 512], in_=ys[:])
            cp_v = lambda o, i: nc.vector.tensor_copy(o, i)
            cp_s = lambda o, i: nc.scalar.copy(o, i)

            def emit_lstm_chunk(l, j, stage_t, gpsh, xres, xbf, nct, fillers=()):
                # Wih*x + bias GEMM straight into PSUM; recurrent matmuls
                # accumulate onto it, activations read PSUM directly.
                # PSUM zero-region is a whole 2KB bank: start=True only on the
                # first write of each bank (gt 0 / gt 8), everything else
                # accumulates -- including the per-step recurrent matmuls.
                for h in range(2):
                    cols = slice(h * 64, (h + 1) * 64)
                    for gt in range(16):
                        nc.tensor.matmul(gpsh[h][:, gt, :], gbias[:, gt, :],
                                         ones_r[:, 0:64], start=(gt % 8 == 0), stop=False,
                                         skip_group_check=True)
                        for ct in range(nct):
                            nc.tensor.matmul(gpsh[h][:, gt, :], wih[:, ct, gt, :],
                                             xbf[:, ct, cols], start=False, stop=(ct == nct - 1),
                                             skip_group_check=True)
                tview = stage_t[:].bitcast(BF16).rearrange("p (c n) -> p c n", c=4)
                fillers = list(fillers)
                for s in range(CS):
                    g = gpsh[s // 8]
                    col = slice((s % 8) * 8, (s % 8) * 8 + 8)
                    for gt in range(16):
                        for ct in range(4):
                            nc.tensor.matmul(g[:, gt, col], whh[:, ct, gt, :], h_bf[:, ct, :],
                                             start=False, stop=(ct == 3), skip_group_check=True)
                        if gt == 7:
                            tg = gw_pool.tile([128, 4, 8], F32, tag="tg")
                            nc.scalar.activation(tg[:], g[:, 0:4, col], AF.Tanh)
                            si = gw_pool.tile([128, 4, 8], F32, tag="si")
                            nc.scalar.activation(si[:], g[:, 4:8, col], AF.Sigmoid)
                            tig = gw_pool.tile([128, 4, 8], F32, tag="tig")
                            nc.vector.tensor_mul(tig[:], si[:], tg[:])
                    sf = gw_pool.tile([128, 4, 8], F32, tag="sf")
                    nc.scalar.activation(sf[:], g[:, 8:12, col], AF.Sigmoid)
                    so = gw_pool.tile([128, 4, 8], F32, tag="so")
                    nc.scalar.activation(so[:], g[:, 12:16, col], AF.Sigmoid)
                    nc.vector.tensor_mul(c_st[:], sf[:], c_st[:])
                    nc.vector.tensor_add(c_st[:], c_st[:], tig[:])
                    tcc = gw_pool.tile([128, 4, 8], F32, tag="tcc")
                    nc.scalar.activation(tcc[:], c_st[:], AF.Tanh)
                    nc.vector.tensor_mul(h_bf[:], so[:], tcc[:])
                    if l < 2:
                        nc.vector.tensor_add(tview[:, :, s * 8:s * 8 + 8], h_bf[:], xres[:, :, s * 8:s * 8 + 8])
                    else:
                        nc.vector.tensor_copy(tview[:, :, s * 8:s * 8 + 8], h_bf[:])
                    if fillers and s % 2 == 1:
                        fillers.pop(0)()
                for f in fillers:
                    f()

            # --------------------------------------------------- tick loop --
            for t in range(NT):
                stage_t = st_pool.tile([128, 256], F32, tag="stage")
                nc.vector.memset(stage_t[:], 0.0)
                gpsh = [gps_pool.tile([128, 16, 64], F32, tag="gps", name="gps%d_%d" % (t, _h))
                        for _h in range(2)]
                do_fc = "fc" in stages and 4 <= t
                if do_fc:
                    jf = t - 4
                    nc.sync.dma_start(out=t3bf[:, :, jf * SB:(jf + 1) * SB],
                                      in_=cco[jf + 3, 2].bitcast(BF16).rearrange("p (c n) -> p c n", c=4)[:, :, 0:SB])
                xres = lw_pool.tile([128, 4, SB], F32, tag="xres")
                xbf = lw_pool.tile([128, 8, SB], BF16, tag="xbf")
                cS = lw_pool.tile([128, 16, 4, CS], F32, tag="cS")
                qpps = qpps_pool.tile([128, 4, 128], F32, tag="qpps")
                qp_all = lw_pool.tile([128, 2, 4, 128], F32, tag="qpall")
                qp_slot = lw_pool.tile([128, 4, 4, CS], F32, tag="qpslot")
                t2b = lw_pool.tile([128, 4, SB], BF16, tag="t2b")

                fill8 = (lambda: [lambda vb=vb: emit_fc_vb(t - 4, vb, cp_v)
                                  for vb in range(8)]) if do_fc else (lambda: [])
                if "l1" in stages and t < NCH:
                    with tc.If(pidc == 0):
                        nc.sync.dma_start(out=xres[:],
                                          in_=di["x_src"].rearrange("c p n -> p c n")[:, :, t * SB:(t + 1) * SB])
                        nc.vector.tensor_copy(xbf[:, 0:4, :], xres[:])
                        emit_lstm_chunk(0, t, stage_t, gpsh, xres, xbf, 4, fill8())
                elif do_fc:
                    with tc.If(pidc == 0):
                        for vb in range(8):
                            emit_fc_vb(t - 4, vb, cp_s)
                if "l2" in stages and 1 <= t < NCH + 1:
                    j = t - 1
                    with tc.If(pidc == 1):
                        nc.sync.dma_start(out=xbf[:, 0:4, :],
                                          in_=cco[j, 0].bitcast(BF16).rearrange("p (c n) -> p c n", c=4)[:, :, 0:SB])
                        nc.vector.tensor_copy(xres[:], xbf[:, 0:4, :])
                        emit_lstm_chunk(1, j, stage_t, gpsh, xres, xbf, 4, fill8())
                elif do_fc:
                    with tc.If(pidc == 1):
                        for vb in range(8):
                            emit_fc_vb(t - 4, vb, cp_s)
                if "l3" in stages and 3 <= t < NCH + 3:
                    j = t - 3
                    with tc.If(pidc == 2):
                        nc.sync.dma_start(out=xbf[:, 0:4, :],
                                          in_=cco[j + 1, 1].bitcast(BF16).rearrange("p (c n) -> p c n", c=4)[:, :, 0:SB])
                        # gather context shards: cores 3..6 have 3 slots, core 7 has 4
                        for k in range(5):
                            nr = 3 if k < 4 else 4
                            pb = 3 * k if k < 4 else 12
                            nc.sync.dma_start(
                                out=cS[:, pb:pb + nr],
                                in_=cco[j + 2, 3 + k][:, 0:nr * 64].rearrange("p (i a q) -> p i a q", i=nr, a=4))
                        csum = lw_pool.tile([128, 8, 4, CS], F32, tag="csum")
                        nc.vector.tensor_add(csum[:], cS[:, 0:8], cS[:, 8:16])
                        for at in range(4):
                            nc.vector.tensor_copy(
                                xbf[:, 4 + at].rearrange("p (s b) -> p s b", b=8),
                                csum[:, :, at, :].rearrange("p b q -> p q b"))
                        emit_lstm_chunk(2, j, stage_t, gpsh, xres, xbf, 8, fill8())

                if "attn" in stages and 2 <= t < NCH + 2:
                    j = t - 2
                    with tc.If(pidc > 2):
                        nc.sync.dma_start(out=t2b[:],
                                          in_=cco[j + 1, 1].bitcast(BF16).rearrange("p (c n) -> p c n", c=4)[:, :, 0:SB])
                        for a in range(2):
                            for at in range(4):
                                for ct in range(4):
                                    nc.tensor.matmul(qpps[:, at, :], qw2[:, a, ct, at, :],
                                                     t2b[:, ct, :], start=(ct == 0), stop=(ct == 3))
                            for at in range(4):
                                nc.scalar.activation(qp_all[:, a, at, :], qpps[:, at, :],
                                                     AF.Identity, bias=qb2[:, a, at:at + 1])
                    for k in range(5):
                        with tc.If(pidv == 3 + k):
                            for i in range(4):
                                a, b = PABS[k][i] if PABS[k][i] else (0, 0)
                                nc.vector.tensor_copy(
                                    qp_slot[:, i],
                                    qp_all[:, a].rearrange("p t (s b) -> p t s b", b=8)[:, :, :, b])
                    with tc.If(pidc > 2):
                        for i in range(4):
                            pw = pwctx_pool.tile([128, 128], F32, tag="pwctx")
                            for q in range(CS):
                                ssum = aw_pool.tile([128, 4, 256], BF16, tag="ssum")
                                stt = aw_pool.tile([128, 4, 256], BF16, tag="stt")
                                for at in range(4):
                                    nc.vector.tensor_scalar_add(ssum[:, at], kp[:, i, at],
                                                                qp_slot[:, i, at, q:q + 1])
                                nc.scalar.activation(stt[:].rearrange("p t k -> p (t k)"),
                                                     ssum[:].rearrange("p t k -> p (t k)"), AF.Tanh)
                                for kb in range(2):
                                    for at in range(4):
                                        nc.tensor.matmul(pw[:, kb * 16 + q:kb * 16 + q + 1],
                                                         stt[:, at, kb * 128:(kb + 1) * 128],
                                                         aww[:, i, at:at + 1],
                                                         start=(at == 0), stop=(at == 3))
                            em = aw_pool.tile([128, 2, CS], BF16, tag="em")
                            nc.scalar.activation(em[:].rearrange("p a b -> p (a b)"), pw[:, 0:32], AF.Exp)
                            emm = aw_pool.tile([128, 2, CS], BF16, tag="emm")
                            nc.vector.tensor_mul(emm[:], em[:],
                                                 amask[:, i, :, j * CS:(j + 1) * CS])
                            for kb in range(2):
                                nc.tensor.matmul(pw[0:1, 48:64], ones_c[:], emm[:, kb],
                                                 start=(kb == 0), stop=(kb == 1))
                            rec = aw_pool.tile([1, CS], F32, tag="rec")
                            nc.vector.reciprocal(rec[:], pw[0:1, 48:64])
                            nc.tensor.matmul(pw[:, 32:48], ones_rf[:], rec[:], start=True, stop=True)
                            wn = aw_pool.tile([128, 2, CS], BF16, tag="wn")
                            for kb in range(2):
                                nc.vector.tensor_mul(wn[:, kb], emm[:, kb], pw[:, 32:48])
                            for at in range(4):
                                for kb in range(2):
                                    nc.tensor.matmul(pw[:, 64 + at * 16:64 + (at + 1) * 16],
                                                     vp[:, i, kb, at * 128:(at + 1) * 128],
                                                     wn[:, kb], start=(kb == 0), stop=(kb == 1))
                            nc.scalar.copy(stage_t[:, i * 64:(i + 1) * 64], pw[:, 64:128])
                            if do_fc and i % 2 == 1:
                                for vb in range((i // 2) * 4, (i // 2) * 4 + 4):
                                    emit_fc_vb(t - 4, vb, cp_v)
                elif do_fc:
                    with tc.If(pidc > 2):
                        for vb in range(8):
                            emit_fc_vb(t - 4, vb, cp_v)

                nc.sync.dma_start(out=cc_in[t].rearrange("(p x) -> p x", p=128), in_=stage_t[:])
                import concourse.mybir as _mb
                nc.gpsimd.collective_compute(
                    "AllGather", _mb.AluOpType.bypass,
                    ins=[cc_in[t]], outs=[cc_out[t]],
                    replica_groups=[list(range(8))])

            # last t3 chunk (produced by final AG) + its fc on all cores
            if "fc" in stages:
                jf = NCH - 1
                nc.sync.dma_start(out=t3bf[:, :, jf * SB:NCH * SB],
                                  in_=cco[NT - 1, 2].bitcast(BF16).rearrange("p (c n) -> p c n", c=4)[:, :, 0:SB])
                for vb in range(8):
                    emit_fc_vb(jf, vb, cp_s)
        if dbg:
            nc.sync.dma_start(out=dbgout.rearrange("t r (p x) -> t r p x", p=128),
                              in_=cc_out.rearrange("t r (p x) -> t r p x", p=128))
    nc.compile()
    _CACHE[key] = nc
    return nc


def kernel(**inputs):
    from concourse.bass_utils import run_bass_kernel_spmd
    nc = build_kernel()
    cores = host_prep(inputs)
    res = run_bass_kernel_spmd(nc, cores, core_ids=list(range(8)))
    return host_post(res.results, inputs)

